# revision 7
# baseline (speedup 1.0000x reference)
"""ColorGAN LUT-lookup kernel for Trainium2 (8 NeuronCores, batch-parallel).

Reference computation (per pixel, per channel c):
    q_c   = (img_c + 1.0) * 127.5
    idx   = int32( q_0*65536 + q_1*256 + q_2 )      # float sum, truncated
    out_c = tanh( weight[idx, c] * img_c + bias[idx, c] )

Sharding: data-parallel over batch (16 images -> 2 per core); the 201MB
weight/bias LUTs are replicated on every core.  Host-side we interleave
weight||bias into one [16M, 6] table so each pixel needs a single 24-byte
indirect-DMA gather instead of two 12-byte ones.

The gather primitive on TRN2 (SWDGE indirect DMA) consumes one offset per
destination partition per instruction: each call gathers 128 rows.
"""

import numpy as np

import concourse.bass as bass
import concourse.mybir as mybir
import concourse.tile as tile
from concourse import bacc
from concourse.bass_utils import run_bass_kernel_spmd

F32 = mybir.dt.float32
I32 = mybir.dt.int32
ALU = mybir.AluOpType
ACTF = mybir.ActivationFunctionType

N_CORES = 8
B, C, H, W = 16, 3, 512, 512
LUT = 256 * 256 * 256
PB = B // N_CORES          # images per core
PLANE = H * W              # 262144 px per plane
P = 128                    # SBUF partitions
K = 512                    # pixels per partition per chunk
CHUNK = P * K              # 65536 px per chunk
NCH_IMG = PLANE // CHUNK   # chunks per image

# exact-fp32 fused constants: ((x+1)*127.5)*65536 == (x+1)*8355840 etc.
# (scaling by 2^16 / 2^8 is exact, so one rounding either way)
SC = [127.5 * 65536.0, 127.5 * 256.0, 127.5]

LAST_RESULTS = None  # test.py introspection


def _compute_idx(nc, io, planes):
    """DVE ops replicating the reference fp32 index arithmetic exactly."""
    s = io.tile([P, K], F32, tag="s")
    tmp = io.tile([P, K], F32, tag="tmp")
    nc.vector.tensor_scalar(out=s[:], in0=planes[0][:], scalar1=1.0,
                            scalar2=SC[0], op0=ALU.add, op1=ALU.mult)
    nc.vector.tensor_scalar(out=tmp[:], in0=planes[1][:], scalar1=1.0,
                            scalar2=SC[1], op0=ALU.add, op1=ALU.mult)
    nc.vector.tensor_tensor(out=s[:], in0=s[:], in1=tmp[:], op=ALU.add)
    nc.vector.tensor_scalar(out=tmp[:], in0=planes[2][:], scalar1=1.0,
                            scalar2=SC[2], op0=ALU.add, op1=ALU.mult)
    nc.vector.tensor_tensor(out=s[:], in0=s[:], in1=tmp[:], op=ALU.add)

    # floor via convert + correct (TRN2 f32->i32 convert rounds to nearest)
    i32 = io.tile([P, K], I32, tag="i32")
    f2 = io.tile([P, K], F32, tag="f2")
    nc.vector.tensor_copy(out=i32[:], in_=s[:])
    nc.vector.tensor_copy(out=f2[:], in_=i32[:])
    nc.vector.tensor_tensor(out=tmp[:], in0=f2[:], in1=s[:], op=ALU.is_gt)
    nc.vector.tensor_tensor(out=f2[:], in0=f2[:], in1=tmp[:], op=ALU.subtract)
    nc.vector.tensor_copy(out=i32[:], in_=f2[:])
    return i32


def _build():
    nc = bacc.Bacc("TRN2", target_bir_lowering=False)
    img = nc.dram_tensor("img", [PB, C, H, W], F32, kind="ExternalInput")
    wb = nc.dram_tensor("wb", [LUT, 6], F32, kind="ExternalInput")
    out = nc.dram_tensor("out", [PB, C, H, W], F32, kind="ExternalOutput")

    img_f = img.rearrange("b c h w -> b c (h w)")
    out_f = out.rearrange("b c h w -> b c (h w)")

    with tile.TileContext(nc) as tc:
        with (
            tc.tile_pool(name="io", bufs=3) as io,
            tc.tile_pool(name="gat", bufs=16) as gat,
        ):
            for b in range(PB):
                for n in range(NCH_IMG):
                    planes = []
                    for c in range(C):
                        src = img_f[b, c].rearrange("(n p k) -> n p k", p=P, k=K)
                        t = io.tile([P, K], F32, tag=f"plane{c}")
                        nc.sync.dma_start(out=t[:], in_=src[n])
                        planes.append(t)

                    i32 = _compute_idx(nc, io, planes)

                    # gather wb[idx]: one indirect DMA per 128-pixel column
                    # (HW limit: 1 offset/partition/call), grouped into
                    # pool-recycled [128, GW*6] tiles for pipelining
                    GW = 64
                    res0 = io.tile([P, K], F32, tag="res0")
                    res1 = io.tile([P, K], F32, tag="res1")
                    res2 = io.tile([P, K], F32, tag="res2")
                    res = [res0, res1, res2]
                    for gi in range(K // GW):
                        g = gat.tile([P, GW * 6], F32, tag="g")
                        for t in range(GW):
                            tt = gi * GW + t
                            nc.gpsimd.indirect_dma_start(
                                out=g[:, t * 6:(t + 1) * 6],
                                out_offset=None,
                                in_=wb[:, :],
                                in_offset=bass.IndirectOffsetOnAxis(
                                    ap=i32[:, tt:tt + 1], axis=0),
                            )
                        gv = g[:].rearrange("p (k s) -> p k s", s=6)
                        sl = slice(gi * GW, (gi + 1) * GW)
                        for c in range(C):
                            nc.vector.tensor_tensor(
                                out=res[c][:, sl], in0=gv[:, :, c],
                                in1=planes[c][:, sl], op=ALU.mult)
                            nc.vector.tensor_tensor(
                                out=res[c][:, sl], in0=res[c][:, sl],
                                in1=gv[:, :, c + 3], op=ALU.add)
                    for c in range(C):
                        nc.scalar.activation(out=res[c][:], in_=res[c][:], func=ACTF.Tanh)
                        dst = out_f[b, c].rearrange("(n p k) -> n p k", p=P, k=K)
                        nc.sync.dma_start(out=dst[n], in_=res[c][:])
    nc.finalize()
    return nc


_NC_CACHE = None


def kernel(img, weight, bias):
    global _NC_CACHE, LAST_RESULTS
    img = np.ascontiguousarray(np.asarray(img, dtype=np.float32))
    weight = np.asarray(weight, dtype=np.float32)
    bias = np.asarray(bias, dtype=np.float32)
    assert img.shape == (B, C, H, W)

    # host-side weight pre-pack: interleave weight||bias rows -> [LUT, 6]
    wb = np.empty((LUT, 6), dtype=np.float32)
    wb[:, 0:3] = weight
    wb[:, 3:6] = bias

    if _NC_CACHE is None:
        _NC_CACHE = _build()
    nc = _NC_CACHE

    import os
    os.environ["BASS_NEVER_TRACE"] = "1"  # no NTFF hook in this container
    in_maps = [
        {"img": img[i * PB:(i + 1) * PB], "wb": wb} for i in range(N_CORES)
    ]
    res = run_bass_kernel_spmd(nc, in_maps, list(range(N_CORES)), trace=False)
    LAST_RESULTS = res
    out = np.concatenate([np.asarray(r["out"]) for r in res.results], axis=0)
    return out
